# revision 4
# baseline (speedup 1.0000x reference)
"""Trainium2 Bass kernel for nn_AttMatch (2-graph attention + SAGEConv GNN).

Self-contained: takes the full unsharded inputs of the reference problem,
shards across 8 NeuronCores internally, runs one SPMD NEFF, and gathers the
full [8192, 8192] sigmoid adjacency output.

Sharding: the concatenated target set (2*4096 rows) is row-sharded across the
8 cores (512 rows of each graph per core).  Key/value and the attention
matrix are sharded along T; the softmax over dim 0 and alpha.T @ v are
completed with a ReduceScatter (global-chunk ordered, so every core receives
exactly its own node chunk).  SAGEConv mean-aggregation is computed as a
dense matmul against the host-built row-normalized adjacency operator,
column-sharded per core; node features are re-replicated with AllGathers.
"""

import numpy as np
import ml_dtypes

import concourse.bass as bass
import concourse.bacc as bacc
import concourse.tile as tile
import concourse.mybir as mybir
from concourse.bass_utils import run_bass_kernel_spmd

BF16 = ml_dtypes.bfloat16

N = 4096          # nodes per graph
D = 128           # feature dim (in == out == 128)
NCORES = 8
SH = N // NCORES  # 512 node shard per graph per core
NI = N // 512     # 8 query chunks of 512
NT = 2 * SH // 128  # 8 local target tiles of 128 (512 of each graph)
NJ = N // 128     # 32 source-node tiles
INV_SCALE = 1.0 / np.sqrt(128.0)

F32 = mybir.dt.float32
BF = mybir.dt.bfloat16

# wm indices (per layer l: base = 7*l)
WK, WQ, WV, WL0, WL1, WR0, WR1 = range(7)
IDENT = 14
# bias indices (per layer l: base = 4*l)
BK, BQ, BV, BL = range(4)

_cache = {}


def _build_nc():
    """Build and compile the SPMD Bass graph (one NeuronCore program)."""
    nc = bacc.Bacc("TRN2", target_bir_lowering=False, debug=False,
                   num_devices=NCORES)

    # ---- external I/O ----
    x1t = nc.dram_tensor("x1t", [D, N], BF, kind="ExternalInput")
    x2t = nc.dram_tensor("x2t", [D, N], BF, kind="ExternalInput")
    xgt_in = [x1t, x2t]
    xown_in = nc.dram_tensor("xown", [2, D, SH], BF, kind="ExternalInput")
    mt_in = [nc.dram_tensor("mt1", [NJ, 128, SH], BF, kind="ExternalInput"),
             nc.dram_tensor("mt2", [NJ, 128, SH], BF, kind="ExternalInput")]
    wm_in = nc.dram_tensor("wm", [15, 128, 128], BF, kind="ExternalInput")
    bs_in = nc.dram_tensor("bs", [8, 128, 1], F32, kind="ExternalInput")
    out_ext = nc.dram_tensor("out", [2, SH, 2 * N], F32, kind="ExternalOutput")

    # ---- internal DRAM for collectives ----
    rg = [list(range(NCORES))]
    rs_in = [[nc.dram_tensor(f"rs_in_{l}_{g}", [NCORES, 129, 512], BF)
              for g in range(2)] for l in range(2)]
    rs_out = [[nc.dram_tensor(f"rs_out_{l}_{g}", [129, 512], BF)
               for g in range(2)] for l in range(2)]
    dag_in = [[nc.dram_tensor(f"dag_in_{l}_{g}", [D, SH], BF)
               for g in range(2)] for l in range(2)]
    dag_out = [[nc.dram_tensor(f"dag_out_{l}_{g}", [NCORES, D, SH], BF,
                               addr_space="Shared")
                for g in range(2)] for l in range(2)]
    hag_in = [[nc.dram_tensor(f"hag_in_{l}_{g}", [D, SH], BF)
               for g in range(2)] for l in range(2)]
    hag_out = [[nc.dram_tensor(f"hag_out_{l}_{g}", [NCORES, D, SH], BF,
                               addr_space="Shared")
                for g in range(2)] for l in range(2)]

    with tile.TileContext(nc) as tc:
        with (
            tc.tile_pool(name="const", bufs=1) as cpool,
            tc.tile_pool(name="xt", bufs=2) as xt_pool,
            tc.tile_pool(name="small", bufs=2) as spool,
            tc.tile_pool(name="kqv", bufs=2) as kqv_pool,
            tc.tile_pool(name="vn", bufs=16) as vn_pool,
            tc.tile_pool(name="es", bufs=12) as es_pool,
            tc.tile_pool(name="csacc", bufs=2) as cs_pool,
            tc.tile_pool(name="stage", bufs=3) as st_pool,
            tc.tile_pool(name="y", bufs=34) as y_pool,
            tc.tile_pool(name="mt", bufs=33) as mt_pool,
            tc.tile_pool(name="z", bufs=6) as z_pool,
            tc.tile_pool(name="ps", bufs=3, space="PSUM") as ps_pool,
            tc.tile_pool(name="ps_p", bufs=2, space="PSUM") as psp_pool,
            tc.tile_pool(name="ps_cs", bufs=2, space="PSUM") as pscs_pool,
        ):
            # ---- load constants ----
            wm = cpool.tile([128, 15 * 128], BF, name="wm_sb")
            for i in range(15):
                nc.scalar.dma_start(wm[:, i * 128:(i + 1) * 128], wm_in[i])
            bs = cpool.tile([128, 8], F32, name="bs_sb")
            for i in range(8):
                nc.scalar.dma_start(bs[:, i:i + 1], bs_in[i])
            ones_m1 = cpool.tile([128, 1], BF, name="ones_m1")
            nc.vector.memset(ones_m1[:], 1.0)
            ones_row = cpool.tile([1, 128], BF, name="ones_row")
            nc.vector.memset(ones_row[:], 1.0)

            def W(l, i):
                base = 7 * l + i if i < 7 else IDENT
                return wm[:, 128 * base:128 * (base + 1)]

            def B(l, i):
                return bs[:, 4 * l + i:4 * l + i + 1]

            ident = wm[:, 128 * IDENT:128 * (IDENT + 1)]

            # ---- load inputs (generation 0) ----
            xgt = []
            for g in range(2):
                t = xt_pool.tile([D, N], BF, name=f"x{g}t_0", tag=f"xt{g}")
                nc.scalar.dma_start(t[:], xgt_in[g][:])
                xgt.append(t)
            xown = []
            for g in range(2):
                t = spool.tile([D, SH], BF, name=f"xown{g}_0", tag=f"xo{g}")
                nc.scalar.dma_start(t[:], xown_in[g])
                xown.append(t)

            hown_final = [None, None]

            for l in range(2):
                # ---- MT prefetch (overlaps attention) ----
                mt_tiles = [[], []]
                for g in range(2):
                    for jt in range(NJ):
                        t = mt_pool.tile([128, SH], BF,
                                         name=f"mt_{l}_{g}_{jt}", tag="mt")
                        nc.scalar.dma_start(t[:], mt_in[g][jt])
                        mt_tiles[g].append(t)

                # ---- projections ----
                kt = kqv_pool.tile([D, 2 * SH], BF, name=f"kt_{l}", tag="kt")
                vnat = []
                for g in range(2):
                    ps = ps_pool.tile([128, 512], F32, tag="ps")
                    nc.tensor.matmul(ps[:], W(l, WK), xown[g][:],
                                     start=True, stop=True)
                    nc.vector.tensor_scalar(kt[:, g * SH:(g + 1) * SH], ps[:],
                                            B(l, BK), None,
                                            mybir.AluOpType.add)
                    # v^T then transpose to natural [t, d] tiles (no bias; bv
                    # is folded in after the softmax division)
                    ps2 = ps_pool.tile([128, 512], F32, tag="ps")
                    nc.tensor.matmul(ps2[:], W(l, WV), xown[g][:],
                                     start=True, stop=True)
                    vt = st_pool.tile([128, SH], BF, name=f"vt_{l}_{g}",
                                      tag="vt", bufs=2)
                    nc.vector.tensor_copy(vt[:], ps2[:])
                    for j in range(4):
                        psy = pscs_pool.tile([128, 128], BF, tag="ps_y",
                                             bufs=2)
                        nc.tensor.transpose(psy[:], vt[:, j * 128:(j + 1) * 128],
                                            ident)
                        vn = vn_pool.tile([128, 128], BF,
                                          name=f"vn_{l}_{g}_{j}", tag="vn")
                        nc.vector.tensor_copy(vn[:], psy[:])
                        vnat.append(vn)
                qt = []
                for g in range(2):
                    q = kqv_pool.tile([D, N], BF, name=f"qt_{l}_{g}",
                                      tag=f"qt{g}")
                    for ic in range(NI):
                        ps = ps_pool.tile([128, 512], F32, tag="ps")
                        nc.tensor.matmul(ps[:], W(l, WQ),
                                         xgt[g][:, ic * 512:(ic + 1) * 512],
                                         start=True, stop=True)
                        nc.vector.tensor_scalar(q[:, ic * 512:(ic + 1) * 512],
                                                ps[:], B(l, BQ), None,
                                                mybir.AluOpType.add)
                    qt.append(q)

                # vnat index: g*4 + j covers target rows of graph g
                dfull = [None, None]
                down = [None, None]
                hown = [None, None]

                for g in range(2):
                    # ---- attention (T-sharded) ----
                    for ic in range(NI):
                        ps_pt = psp_pool.tile([128, 512], F32, tag="ps_p")
                        csa = cs_pool.tile([128, 512], BF, tag="cs")
                        for tt in range(NT):
                            ps_s = ps_pool.tile([128, 512], F32, tag="ps")
                            nc.tensor.matmul(
                                ps_s[:],
                                kt[:, tt * 128:(tt + 1) * 128],
                                qt[g][:, ic * 512:(ic + 1) * 512],
                                start=True, stop=True)
                            es = es_pool.tile([128, 512], BF, tag="es")
                            nc.scalar.activation(
                                es[:], ps_s[:],
                                mybir.ActivationFunctionType.Exp,
                                scale=INV_SCALE)
                            nc.tensor.matmul(ps_pt[:], vnat[tt], es[:],
                                             start=(tt == 0), stop=(tt == NT - 1))
                            if tt == 0:
                                nc.vector.tensor_copy(csa[:], es[:])
                            else:
                                nc.vector.tensor_tensor(
                                    csa[:], csa[:], es[:],
                                    mybir.AluOpType.add)
                        ps_c = pscs_pool.tile([1, 512], F32, tag="ps_cs", bufs=1)
                        nc.tensor.matmul(ps_c[:], ones_m1[:], csa[:],
                                         start=True, stop=True)
                        pc = st_pool.tile([128, 512], BF, tag="pc")
                        nc.vector.tensor_copy(pc[:], ps_pt[:])
                        cc = st_pool.tile([1, 512], BF, tag="cc")
                        nc.vector.tensor_copy(cc[:], ps_c[:])
                        nc.scalar.dma_start(rs_in[l][g][ic, 0:128, :], pc[:])
                        nc.scalar.dma_start(rs_in[l][g][ic, 128:129, :], cc[:])
                    nc.gpsimd.collective_compute(
                        "ReduceScatter", mybir.AluOpType.add,
                        replica_groups=rg,
                        ins=[rs_in[l][g][:]], outs=[rs_out[l][g][:]])

                    # ---- softmax epilogue on own chunk ----
                    pown = spool.tile([128, 512], BF, name=f"pown_{l}_{g}",
                                      tag="pown")
                    nc.scalar.dma_start(pown[:], rs_out[l][g][0:128, :])
                    csb = spool.tile([1, 512], BF, name=f"csb_{l}_{g}",
                                     tag="csb")
                    nc.scalar.dma_start(csb[:], rs_out[l][g][128:129, :])
                    csf = spool.tile([1, 512], F32, name=f"csf_{l}_{g}",
                                     tag="csf")
                    nc.vector.tensor_copy(csf[:], csb[:])
                    rcs = spool.tile([1, 512], F32, name=f"rcs_{l}_{g}",
                                     tag="rcs")
                    nc.vector.reciprocal(rcs[:], csf[:])
                    rcsb = spool.tile([1, 512], BF, name=f"rcsb_{l}_{g}",
                                      tag="rcsb")
                    nc.vector.tensor_copy(rcsb[:], rcs[:])
                    ps_rep = ps_pool.tile([128, 512], F32, tag="ps")
                    nc.tensor.matmul(ps_rep[:], ones_row[:], rcsb[:],
                                     start=True, stop=True)
                    prod = spool.tile([128, 512], BF, name=f"prod_{l}_{g}",
                                      tag="prod")
                    nc.vector.tensor_tensor(prod[:], pown[:], ps_rep[:],
                                            mybir.AluOpType.mult)
                    t2 = spool.tile([128, 512], BF, name=f"t2_{l}_{g}",
                                    tag="t2")
                    nc.vector.tensor_tensor(t2[:], xown[g][:], prod[:],
                                            mybir.AluOpType.subtract)
                    dn = spool.tile([128, 512], BF, name=f"down_{l}_{g}",
                                    tag=f"down{g}")
                    nc.vector.tensor_scalar(dn[:], t2[:], B(l, BV), None,
                                            mybir.AluOpType.subtract)
                    down[g] = dn
                    nc.scalar.dma_start(dag_in[l][g][:], dn[:])
                    nc.gpsimd.collective_compute(
                        "AllGather", mybir.AluOpType.bypass,
                        replica_groups=rg,
                        ins=[dag_in[l][g][:]], outs=[dag_out[l][g][:]])
                    df = xt_pool.tile([D, N], BF, name=f"dfull_{l}_{g}",
                                      tag=f"df{g}", bufs=1)
                    for c in range(NCORES):
                        nc.scalar.dma_start(df[:, c * SH:(c + 1) * SH],
                                            dag_out[l][g][c])
                    dfull[g] = df

                for g in range(2):
                    # ---- SAGE ----
                    ytiles = []
                    for jt in range(NJ):
                        psy = pscs_pool.tile([128, 128], F32, tag="ps_y",
                                             bufs=2)
                        nc.tensor.matmul(psy[:],
                                         xgt[g][:, jt * 128:(jt + 1) * 128],
                                         W(l, WL0), start=True, stop=False)
                        nc.tensor.matmul(psy[:],
                                         dfull[g][:, jt * 128:(jt + 1) * 128],
                                         W(l, WL1), start=False, stop=True)
                        y = y_pool.tile([128, 128], BF,
                                        name=f"y_{l}_{g}_{jt}", tag="y")
                        nc.vector.tensor_copy(y[:], psy[:])
                        ytiles.append(y)
                    ps_a = psp_pool.tile([128, 512], F32, tag="ps_p")
                    for jt in range(NJ):
                        nc.tensor.matmul(ps_a[:], ytiles[jt][:],
                                         mt_tiles[g][jt][:],
                                         start=(jt == 0), stop=False)
                    nc.tensor.matmul(ps_a[:], W(l, WR0), xown[g][:],
                                     start=False, stop=False)
                    nc.tensor.matmul(ps_a[:], W(l, WR1), down[g][:],
                                     start=False, stop=True)
                    h = spool.tile([D, SH], BF, name=f"hown_{l}_{g}",
                                   tag=f"xo{g}")
                    if l == 0:
                        nc.vector.tensor_scalar(
                            h[:], ps_a[:], B(l, BL), 0.0,
                            mybir.AluOpType.add, mybir.AluOpType.max)
                    else:
                        nc.vector.tensor_scalar(
                            h[:], ps_a[:], B(l, BL), None,
                            mybir.AluOpType.add)
                    hown[g] = h
                    nc.scalar.dma_start(hag_in[l][g][:], h[:])
                    nc.gpsimd.collective_compute(
                        "AllGather", mybir.AluOpType.bypass,
                        replica_groups=rg,
                        ins=[hag_in[l][g][:]], outs=[hag_out[l][g][:]])

                # ---- gather new X generation ----
                new_xgt = []
                for g in range(2):
                    t = xt_pool.tile([D, N], BF, name=f"x{g}t_{l + 1}",
                                     tag=f"xt{g}")
                    for c in range(NCORES):
                        nc.scalar.dma_start(t[:, c * SH:(c + 1) * SH],
                                            hag_out[l][g][c])
                    new_xgt.append(t)
                xgt = new_xgt
                xown = hown
                if l == 1:
                    hown_final = hown

            # ---- final adjacency: sigmoid(F @ F^T), own 1024 rows ----
            for g in range(2):
                for rt in range(4):
                    lhs = hown_final[g][:, rt * 128:(rt + 1) * 128]
                    for cb in range(16):
                        src = xgt[cb // 8]
                        ps_z = ps_pool.tile([128, 512], F32, tag="ps")
                        nc.tensor.matmul(
                            ps_z[:], lhs,
                            src[:, (cb % 8) * 512:(cb % 8 + 1) * 512],
                            start=True, stop=True)
                        z = z_pool.tile([128, 512], F32, tag="z")
                        nc.scalar.activation(
                            z[:], ps_z[:],
                            mybir.ActivationFunctionType.Sigmoid)
                        nc.scalar.dma_start(
                            out_ext[g, rt * 128:(rt + 1) * 128,
                                    cb * 512:(cb + 1) * 512],
                            z[:])

    nc.compile()
    return nc


def _host_prep(inputs):
    """Build per-core input maps from the full problem inputs."""
    x1 = np.asarray(inputs["x1"], np.float32)
    x2 = np.asarray(inputs["x2"], np.float32)
    x1t = np.ascontiguousarray(x1.T).astype(BF16)
    x2t = np.ascontiguousarray(x2.T).astype(BF16)

    def norm_adj_t(ei):
        ei = np.asarray(ei)
        A = np.zeros((N, N), np.float32)
        np.add.at(A, (ei[1], ei[0]), 1.0)
        deg = A.sum(1)
        A /= np.maximum(deg, 1.0)[:, None]
        return np.ascontiguousarray(A.T)  # MT[j, n]

    mt = [norm_adj_t(inputs["ei1"]), norm_adj_t(inputs["ei2"])]

    wm = np.zeros((15, 128, 128), np.float32)
    bs = np.zeros((8, 128, 1), np.float32)
    for l, s in enumerate(("1", "2")):
        wm[7 * l + WK] = inputs["Wk" + s]
        wm[7 * l + WQ] = inputs["Wq" + s]
        wm[7 * l + WV] = inputs["Wv" + s]
        wm[7 * l + WL0] = inputs["Wl" + s][:128]
        wm[7 * l + WL1] = inputs["Wl" + s][128:]
        wm[7 * l + WR0] = inputs["Wr" + s][:128]
        wm[7 * l + WR1] = inputs["Wr" + s][128:]
        bs[4 * l + BK, :, 0] = inputs["bk" + s]
        bs[4 * l + BQ, :, 0] = inputs["bq" + s]
        bs[4 * l + BV, :, 0] = inputs["bv" + s]
        bs[4 * l + BL, :, 0] = inputs["bl" + s]
    wm[IDENT] = np.eye(128)
    wm = wm.astype(BF16)

    in_maps = []
    for c in range(NCORES):
        sl = slice(c * SH, (c + 1) * SH)
        in_maps.append({
            "x1t": x1t,
            "x2t": x2t,
            "xown": np.stack([x1t[:, sl], x2t[:, sl]]),
            "mt1": np.ascontiguousarray(
                mt[0][:, sl].astype(BF16).reshape(NJ, 128, SH)),
            "mt2": np.ascontiguousarray(
                mt[1][:, sl].astype(BF16).reshape(NJ, 128, SH)),
            "wm": wm,
            "bs": bs,
        })
    return in_maps


def _assemble(results):
    full = np.empty((2 * N, 2 * N), np.float32)
    for c in range(NCORES):
        o = results[c]["out"]
        full[c * SH:(c + 1) * SH] = o[0]
        full[N + c * SH:N + (c + 1) * SH] = o[1]
    return full


def get_nc():
    if "nc" not in _cache:
        _cache["nc"] = _build_nc()
    return _cache["nc"]


def kernel(**inputs):
    nc = get_nc()
    in_maps = _host_prep(inputs)
    res = run_bass_kernel_spmd(nc, in_maps, core_ids=list(range(NCORES)))
    return _assemble(res.results)


# revision 10
# speedup vs baseline: 1.0360x; 1.0360x over previous
"""Trainium2 Bass kernel for nn_AttMatch (2-graph attention + SAGEConv GNN).

Self-contained: takes the full unsharded inputs of the reference problem,
shards across 8 NeuronCores internally, runs one SPMD NEFF, and gathers the
full [8192, 8192] sigmoid adjacency output.

Sharding: the concatenated target set (2*4096 rows) is row-sharded across the
8 cores (512 rows of each graph per core).  Key/value and the attention
matrix are sharded along T; the softmax over dim 0 and alpha.T @ v are
completed with a ReduceScatter (global-chunk ordered, so every core receives
exactly its own node chunk).  SAGEConv mean-aggregation is computed as a
dense matmul against the host-built row-normalized adjacency operator,
column-sharded per core; node features are re-replicated with AllGathers.
"""

import numpy as np
import ml_dtypes

import concourse.bass as bass
import concourse.bacc as bacc
import concourse.tile as tile
import concourse.mybir as mybir
from concourse.bass_utils import run_bass_kernel_spmd

BF16 = ml_dtypes.bfloat16

N = 4096          # nodes per graph
D = 128           # feature dim (in == out == 128)
NCORES = 8
SH = N // NCORES  # 512 node shard per graph per core
ICW = 1024        # query-chunk width
NIC = N // ICW    # 4 query chunks
NT = 2 * SH // 128  # 8 local target tiles of 128 (512 of each graph)
NJ = N // 128     # 32 source-node tiles
INV_SCALE = 1.0 / np.sqrt(128.0)

F32 = mybir.dt.float32
BF = mybir.dt.bfloat16

ADD = mybir.AluOpType.add
SUB = mybir.AluOpType.subtract
MULT = mybir.AluOpType.mult
MAX = mybir.AluOpType.max

# wm indices (per layer l: base = 7*l)
WK, WQ, WV, WL0, WL1, WR0, WR1 = range(7)
IDENT = 14
# bias indices (per layer l: base = 4*l)
BK, BQ, BV, BL = range(4)

_cache = {}


def _build_nc():
    """Build and compile the SPMD Bass graph (one NeuronCore program)."""
    nc = bacc.Bacc("TRN2", target_bir_lowering=False, debug=False,
                   num_devices=NCORES)

    # ---- external I/O ----
    x1t = nc.dram_tensor("x1t", [D, N], BF, kind="ExternalInput")
    x2t = nc.dram_tensor("x2t", [D, N], BF, kind="ExternalInput")
    xgt_in = [x1t, x2t]
    xown_in = nc.dram_tensor("xown", [2, D, SH], BF, kind="ExternalInput")
    mt_in = [nc.dram_tensor("mt1", [NJ, 128, SH], BF, kind="ExternalInput"),
             nc.dram_tensor("mt2", [NJ, 128, SH], BF, kind="ExternalInput")]
    wm_in = nc.dram_tensor("wm", [15, 128, 128], BF, kind="ExternalInput")
    bs_in = nc.dram_tensor("bs", [8, 128, 1], F32, kind="ExternalInput")
    out_ext = nc.dram_tensor("out", [2, SH, 2 * N], F32, kind="ExternalOutput")

    # ---- internal DRAM for collectives ----
    rg = [list(range(NCORES))]
    rs_in = [[nc.dram_tensor(f"rs_in_{l}_{g}", [NCORES, 129, 512], BF)
              for g in range(2)] for l in range(2)]
    rs_out = [[nc.dram_tensor(f"rs_out_{l}_{g}", [129, 512], BF)
               for g in range(2)] for l in range(2)]
    dag_in = [[nc.dram_tensor(f"dag_in_{l}_{g}", [D, SH], BF)
               for g in range(2)] for l in range(2)]
    dag_out = [[nc.dram_tensor(f"dag_out_{l}_{g}", [NCORES, D, SH], BF,
                               addr_space="Shared")
                for g in range(2)] for l in range(2)]
    hag_in = [[nc.dram_tensor(f"hag_in_{l}_{g}", [D, SH], BF)
               for g in range(2)] for l in range(2)]
    hag_out = [[nc.dram_tensor(f"hag_out_{l}_{g}", [NCORES, D, SH], BF,
                               addr_space="Shared")
                for g in range(2)] for l in range(2)]

    with tile.TileContext(nc) as tc:
        with (
            tc.tile_pool(name="const", bufs=1) as cpool,
            tc.tile_pool(name="xt", bufs=2) as xt_pool,
            tc.tile_pool(name="small", bufs=2) as spool,
            tc.tile_pool(name="kqv", bufs=1) as kqv_pool,
            tc.tile_pool(name="es", bufs=4) as es_pool,
            tc.tile_pool(name="csacc", bufs=2) as cs_pool,
            tc.tile_pool(name="stage", bufs=3) as st_pool,
            tc.tile_pool(name="ybig", bufs=1) as y_pool,
            tc.tile_pool(name="mt", bufs=1) as mt_pool,
            tc.tile_pool(name="z", bufs=2) as z_pool,
            tc.tile_pool(name="ps", bufs=2, space="PSUM") as ps_pool,
            tc.tile_pool(name="ps_p", bufs=3, space="PSUM") as psp_pool,
            tc.tile_pool(name="ps_cs", bufs=1, space="PSUM") as pscs_pool,
        ):
            # ---- load constants ----
            wm = cpool.tile([128, 15 * 128], BF, name="wm_sb")
            nc.scalar.dma_start(
                wm.rearrange("p (i f) -> p i f", i=15),
                wm_in.ap().rearrange("i p f -> p i f"))
            bs = cpool.tile([128, 8], F32, name="bs_sb")
            nc.scalar.dma_start(
                bs.rearrange("p (i f) -> p i f", i=8),
                bs_in.ap().rearrange("i p f -> p i f"))
            ones_m1 = cpool.tile([128, 1], BF, name="ones_m1")
            nc.vector.memset(ones_m1[:], 1.0)
            ones_row = cpool.tile([1, 128], BF, name="ones_row")
            nc.vector.memset(ones_row[:], 1.0)

            def W(l, i):
                base = 7 * l + i if i < 7 else IDENT
                return wm[:, 128 * base:128 * (base + 1)]

            def B(l, i):
                return bs[:, 4 * l + i:4 * l + i + 1]

            ident = wm[:, 128 * IDENT:128 * (IDENT + 1)]

            # ---- load inputs (generation 0) ----
            xgt = []
            for g in range(2):
                t = xt_pool.tile([D, N], BF, name=f"x{g}t_0", tag=f"xt{g}")
                nc.scalar.dma_start(t[:], xgt_in[g][:])
                xgt.append(t)
            xown = []
            for g in range(2):
                t = spool.tile([D, SH], BF, name=f"xown{g}_0", tag=f"xo{g}")
                nc.scalar.dma_start(t[:], xown_in[g])
                xown.append(t)

            hown_final = [None, None]

            for l in range(2):
                # ---- MT loads (overlap attention) ----
                mt_big = []
                for g in range(2):
                    t = mt_pool.tile([128, NJ * SH], BF,
                                     name=f"mt_{l}_{g}", tag=f"mt{g}")
                    nc.scalar.dma_start(
                        t.rearrange("p (j n) -> p j n", j=NJ),
                        mt_in[g].ap().rearrange("j p n -> p j n"))
                    mt_big.append(t)

                # ---- projections ----
                kt = kqv_pool.tile([D, 2 * SH], BF, name=f"kt_{l}", tag="kt",
                                   bufs=2)
                vnat = []
                for g in range(2):
                    ps = psp_pool.tile([128, 512], F32, tag="ps_p")
                    nc.tensor.matmul(ps[:], W(l, WK), xown[g][:],
                                     start=True, stop=True)
                    nc.vector.tensor_scalar(kt[:, g * SH:(g + 1) * SH], ps[:],
                                            B(l, BK), None, ADD)
                    # v^T then transpose to natural [t, d] tiles (no bias; bv
                    # is folded in after the softmax division)
                    ps2 = psp_pool.tile([128, 512], F32, tag="ps_p")
                    nc.tensor.matmul(ps2[:], W(l, WV), xown[g][:],
                                     start=True, stop=True)
                    vt = st_pool.tile([128, SH], BF, name=f"vt_{l}_{g}",
                                      tag="vt", bufs=2)
                    nc.vector.tensor_copy(vt[:], ps2[:])
                    psv = psp_pool.tile([128, 512], BF, tag="ps_p")
                    for j in range(4):
                        nc.tensor.transpose(psv[:, j * 128:(j + 1) * 128],
                                            vt[:, j * 128:(j + 1) * 128],
                                            ident)
                    vb = st_pool.tile([128, 512], BF, name=f"vn_{l}_{g}",
                                      tag=f"vn{g}", bufs=1)
                    nc.vector.tensor_copy(vb[:], psv[:])
                    vnat += [vb[:, j * 128:(j + 1) * 128] for j in range(4)]
                qt = []
                for g in range(2):
                    q = kqv_pool.tile([D, N], BF, name=f"qt_{l}_{g}",
                                      tag=f"qt{g}", bufs=1)
                    for ic in range(NIC):
                        ps = ps_pool.tile([128, ICW], F32, tag="ps")
                        for h in range(2):
                            nc.tensor.matmul(
                                ps[:, h * 512:(h + 1) * 512], W(l, WQ),
                                xgt[g][:, ic * ICW + h * 512:
                                        ic * ICW + (h + 1) * 512],
                                start=True, stop=True)
                        nc.vector.tensor_scalar(q[:, ic * ICW:(ic + 1) * ICW],
                                                ps[:], B(l, BQ), None, ADD)
                    qt.append(q)

                dfull = [None, None]
                down = [None, None]
                hown = [None, None]

                def attention(g):
                    for ic in range(NIC):
                        php = [psp_pool.tile([128, 512], F32, tag="ps_p",
                                             name=f"php{h}_{l}_{g}_{ic}")
                               for h in range(2)]
                        csa = cs_pool.tile([128, ICW], BF, tag="cs")
                        estiles = []
                        for tt in range(NT):
                            ps_s = ps_pool.tile([128, ICW], F32, tag="ps")
                            for h in range(2):
                                nc.tensor.matmul(
                                    ps_s[:, h * 512:(h + 1) * 512],
                                    kt[:, tt * 128:(tt + 1) * 128],
                                    qt[g][:, ic * ICW + h * 512:
                                            ic * ICW + (h + 1) * 512],
                                    start=True, stop=True)
                            es = es_pool.tile([128, ICW], BF, tag="es")
                            nc.scalar.activation(
                                es[:], ps_s[:],
                                mybir.ActivationFunctionType.Exp,
                                scale=INV_SCALE)
                            estiles.append(es)
                            for h in range(2):
                                nc.tensor.matmul(
                                    php[h][:], vnat[tt],
                                    es[:, h * 512:(h + 1) * 512],
                                    start=(tt == 0), stop=(tt == NT - 1))
                            if tt == 0:
                                nc.vector.tensor_copy(csa[:], es[:])
                            else:
                                nc.vector.tensor_tensor(csa[:], csa[:], es[:],
                                                        ADD)
                        pc = st_pool.tile([128, ICW], BF, tag="pc")
                        cc = st_pool.tile([1, ICW], BF, tag="cc")
                        for h in range(2):
                            nc.vector.tensor_copy(
                                pc[:, h * 512:(h + 1) * 512], php[h][:])
                            ps_c = pscs_pool.tile([1, 512], F32, tag="ps_cs")
                            nc.tensor.matmul(ps_c[:], ones_m1[:],
                                             csa[:, h * 512:(h + 1) * 512],
                                             start=True, stop=True)
                            nc.vector.tensor_copy(
                                cc[:, h * 512:(h + 1) * 512], ps_c[:])
                        nc.scalar.dma_start(
                            rs_in[l][g][2 * ic:2 * ic + 2, 0:128, :]
                            .rearrange("c p n -> p c n"),
                            pc.rearrange("p (c n) -> p c n", c=2))
                        nc.scalar.dma_start(
                            rs_in[l][g][2 * ic:2 * ic + 2, 128:129, :]
                            .rearrange("c p n -> p c n"),
                            cc.rearrange("p (c n) -> p c n", c=2))
                    nc.gpsimd.collective_compute(
                        "ReduceScatter", ADD, replica_groups=rg,
                        ins=[rs_in[l][g][:]], outs=[rs_out[l][g][:]])

                def epilogue(g):
                    """Own-chunk softmax finish + D AllGather."""
                    pown = spool.tile([128, 512], BF, name=f"pown_{l}_{g}",
                                      tag="pown")
                    nc.scalar.dma_start(pown[:], rs_out[l][g][0:128, :])
                    csb = spool.tile([1, 512], BF, name=f"csb_{l}_{g}",
                                     tag="csb")
                    nc.scalar.dma_start(csb[:], rs_out[l][g][128:129, :])
                    ps_rep = psp_pool.tile([128, 512], F32, tag="ps_p")
                    nc.tensor.matmul(ps_rep[:], ones_row[:], csb[:],
                                     start=True, stop=True)
                    rrep = spool.tile([128, 512], BF, name=f"rrep_{l}_{g}",
                                      tag="rrep")
                    with nc.allow_low_precision(reason="saturated sigmoid output tolerates bf16"):
                        nc.vector.reciprocal(rrep[:], ps_rep[:])
                    prod = spool.tile([128, 512], BF, name=f"prod_{l}_{g}",
                                      tag="prod")
                    nc.vector.tensor_tensor(prod[:], pown[:], rrep[:], MULT)
                    t2 = spool.tile([128, 512], BF, name=f"t2_{l}_{g}",
                                    tag="t2")
                    nc.vector.tensor_tensor(t2[:], xown[g][:], prod[:], SUB)
                    dn = spool.tile([128, 512], BF, name=f"down_{l}_{g}",
                                    tag=f"down{g}")
                    nc.vector.tensor_scalar(dn[:], t2[:], B(l, BV), None, SUB)
                    down[g] = dn
                    nc.scalar.dma_start(dag_in[l][g][:], dn[:])
                    nc.gpsimd.collective_compute(
                        "AllGather", mybir.AluOpType.bypass, replica_groups=rg,
                        ins=[dag_in[l][g][:]], outs=[dag_out[l][g][:]])
                    df = xt_pool.tile([D, N], BF, name=f"dfull_{l}_{g}",
                                      tag=f"df{g}", bufs=1)
                    nc.scalar.dma_start(
                        df.rearrange("p (c n) -> p c n", c=NCORES),
                        dag_out[l][g].ap().rearrange("c p n -> p c n"))
                    dfull[g] = df

                def sage(g):
                    ybig = y_pool.tile([128, NJ * 128], BF,
                                       name=f"y_{l}_{g}", tag=f"y{g}")
                    for jb in range(NJ // 4):
                        psy = psp_pool.tile([128, 512], F32, tag="ps_p")
                        for k in range(4):
                            jt = jb * 4 + k
                            sl = slice(jt * 128, (jt + 1) * 128)
                            nc.tensor.matmul(psy[:, k * 128:(k + 1) * 128],
                                             xgt[g][:, sl], W(l, WL0),
                                             start=True, stop=False)
                            nc.tensor.matmul(psy[:, k * 128:(k + 1) * 128],
                                             dfull[g][:, sl], W(l, WL1),
                                             start=False, stop=True)
                        nc.vector.tensor_copy(
                            ybig[:, jb * 512:(jb + 1) * 512], psy[:])
                    ps_a = psp_pool.tile([128, 512], F32, tag="ps_p")
                    for jt in range(NJ):
                        nc.tensor.matmul(
                            ps_a[:], ybig[:, jt * 128:(jt + 1) * 128],
                            mt_big[g][:, jt * SH:(jt + 1) * SH],
                            start=(jt == 0), stop=False)
                    nc.tensor.matmul(ps_a[:], W(l, WR0), xown[g][:],
                                     start=False, stop=False)
                    nc.tensor.matmul(ps_a[:], W(l, WR1), down[g][:],
                                     start=False, stop=True)
                    h = spool.tile([D, SH], BF, name=f"hown_{l}_{g}",
                                   tag=f"xo{g}")
                    if l == 0:
                        nc.vector.tensor_scalar(h[:], ps_a[:], B(l, BL), 0.0,
                                                ADD, MAX)
                    else:
                        nc.vector.tensor_scalar(h[:], ps_a[:], B(l, BL), None,
                                                ADD)
                    hown[g] = h
                    nc.scalar.dma_start(hag_in[l][g][:], h[:])
                    nc.gpsimd.collective_compute(
                        "AllGather", mybir.AluOpType.bypass, replica_groups=rg,
                        ins=[hag_in[l][g][:]], outs=[hag_out[l][g][:]])

                attention(0)
                attention(1)
                epilogue(0)
                sage(0)
                epilogue(1)
                sage(1)

                # ---- gather new X generation ----
                new_xgt = []
                for g in range(2):
                    t = xt_pool.tile([D, N], BF, name=f"x{g}t_{l + 1}",
                                     tag=f"xt{g}")
                    nc.scalar.dma_start(
                        t.rearrange("p (c n) -> p c n", c=NCORES),
                        hag_out[l][g].ap().rearrange("c p n -> p c n"))
                    new_xgt.append(t)
                xgt = new_xgt
                xown = hown
                if l == 1:
                    hown_final = hown

            # ---- final adjacency: sigmoid(F @ F^T), own 1024 rows ----
            for g in range(2):
                for rt in range(4):
                    lhs = hown_final[g][:, rt * 128:(rt + 1) * 128]
                    for cb in range(8):
                        src = xgt[cb // 4]
                        c0 = (cb % 4) * ICW
                        ps_z = ps_pool.tile([128, ICW], F32, tag="ps")
                        for h in range(2):
                            nc.tensor.matmul(
                                ps_z[:, h * 512:(h + 1) * 512], lhs,
                                src[:, c0 + h * 512:c0 + (h + 1) * 512],
                                start=True, stop=True)
                        z = z_pool.tile([128, ICW], F32, tag="z")
                        nc.scalar.activation(
                            z[:], ps_z[:],
                            mybir.ActivationFunctionType.Sigmoid)
                        nc.gpsimd.dma_start(
                            out_ext[g, rt * 128:(rt + 1) * 128,
                                    cb * ICW:(cb + 1) * ICW],
                            z[:])

    nc.compile()
    return nc


def _host_prep(inputs):
    """Build per-core input maps from the full problem inputs."""
    x1 = np.asarray(inputs["x1"], np.float32)
    x2 = np.asarray(inputs["x2"], np.float32)
    x1t = np.ascontiguousarray(x1.T).astype(BF16)
    x2t = np.ascontiguousarray(x2.T).astype(BF16)

    def norm_adj_t(ei):
        ei = np.asarray(ei)
        A = np.zeros((N, N), np.float32)
        np.add.at(A, (ei[1], ei[0]), 1.0)
        deg = A.sum(1)
        A /= np.maximum(deg, 1.0)[:, None]
        return np.ascontiguousarray(A.T)  # MT[j, n]

    mt = [norm_adj_t(inputs["ei1"]), norm_adj_t(inputs["ei2"])]

    wm = np.zeros((15, 128, 128), np.float32)
    bs = np.zeros((8, 128, 1), np.float32)
    for l, s in enumerate(("1", "2")):
        wm[7 * l + WK] = inputs["Wk" + s]
        wm[7 * l + WQ] = inputs["Wq" + s]
        wm[7 * l + WV] = inputs["Wv" + s]
        wm[7 * l + WL0] = inputs["Wl" + s][:128]
        wm[7 * l + WL1] = inputs["Wl" + s][128:]
        wm[7 * l + WR0] = inputs["Wr" + s][:128]
        wm[7 * l + WR1] = inputs["Wr" + s][128:]
        bs[4 * l + BK, :, 0] = inputs["bk" + s]
        bs[4 * l + BQ, :, 0] = inputs["bq" + s]
        bs[4 * l + BV, :, 0] = inputs["bv" + s]
        bs[4 * l + BL, :, 0] = inputs["bl" + s]
    wm[IDENT] = np.eye(128)
    wm = wm.astype(BF16)

    in_maps = []
    for c in range(NCORES):
        sl = slice(c * SH, (c + 1) * SH)
        in_maps.append({
            "x1t": x1t,
            "x2t": x2t,
            "xown": np.stack([x1t[:, sl], x2t[:, sl]]),
            "mt1": np.ascontiguousarray(
                mt[0][:, sl].astype(BF16).reshape(NJ, 128, SH)),
            "mt2": np.ascontiguousarray(
                mt[1][:, sl].astype(BF16).reshape(NJ, 128, SH)),
            "wm": wm,
            "bs": bs,
        })
    return in_maps


def _assemble(results):
    full = np.empty((2 * N, 2 * N), np.float32)
    for c in range(NCORES):
        o = results[c]["out"]
        full[c * SH:(c + 1) * SH] = o[0]
        full[N + c * SH:N + (c + 1) * SH] = o[1]
    return full


def get_nc():
    if "nc" not in _cache:
        _cache["nc"] = _build_nc()
    return _cache["nc"]


def kernel(**inputs):
    nc = get_nc()
    in_maps = _host_prep(inputs)
    res = run_bass_kernel_spmd(nc, in_maps, core_ids=list(range(NCORES)))
    return _assemble(res.results)
